# revision 5
# baseline (speedup 1.0000x reference)
"""Trainium2 Bass kernel for a sparse-attention encoder layer.

Model: B=4, L=4096, D=1024, H=16 heads (HD=64), chunked local attention with
window W=256 (each chunk of 256 queries attends to its own chunk plus the
previous chunk), FFN 1024->4096->1024 with ReLU, two post-residual layernorms.

Sharding: sequence-parallel over 8 cores. Flattened (B*L = 16384) tokens are
split into 8 contiguous shards of 2048 tokens (core c gets batch c//2, half
c%2). Each core receives its own tokens plus a 256-token halo (the preceding
chunk, used only as keys/values); batch-start shards get a zero halo plus an
additive -1e9 score mask for those key slots, exactly matching the reference.
No inter-core communication is needed.

Per-core pipeline (matmuls bf16 with fp32 PSUM accumulation; softmax,
layernorm and residuals in fp32):
  per chunk: QKV projections -> local attention (token-major softmax on the
  free axis, no max subtraction -- scores are bounded, |s|<3; PE-transpose of
  the normalized attention matrix; AV matmul with 2-head column packing) ->
  output projection -> residual + LN1 -> x1 to DRAM.
  Then FFN pass 1 (h = relu(x1@w1+b) streamed to DRAM) and FFN pass 2
  (h@w2 accumulated in PSUM over all 4096 hidden, + residual + LN2) -> out.
"""

import os
import sys

sys.path.insert(0, "/opt/trn_rl_repo")

import ml_dtypes
import numpy as np

import concourse.bacc as bacc
import concourse.bass as bass
import concourse.mybir as mybir
import concourse.tile as tile
from concourse.bass_utils import run_bass_kernel_spmd
from concourse.masks import make_identity

BF16 = mybir.dt.bfloat16
F32 = mybir.dt.float32
AF = mybir.ActivationFunctionType
ALU = mybir.AluOpType
AX = mybir.AxisListType

NCORES = 8
D = 1024
H = 16
HD = 64
W = 256
FF = 4096
TOWN = 2048          # own tokens per core
TEXT = TOWN + W      # with halo
NCH = int(os.environ.get("BASS_NCH", TOWN // W))  # own chunks per core
KD = D // 128        # 8  (contraction tiles over D)
KF = FF // 128       # 32
EPS = 1e-6
SCALE = 1.0 / float(np.sqrt(HD))


def _layernorm(nc, pool, resid, g_b, eps_ap, out1, dst1, out2=None, dst2=None):
    """LayerNorm of resid [128, D] (stats over the free axis); writes
    (resid-mean)*rstd*gamma + beta to DRAM for each (name, dtype, beta) out."""
    st = pool.tile([128, 8], F32, tag="ln_stats", bufs=3)
    tsum, nm, ssq, m2, var, stdv, rstd = (st[:, i:i + 1] for i in range(7))
    nc.vector.tensor_reduce(tsum, resid, axis=AX.X, op=ALU.add)
    nc.vector.tensor_scalar_mul(nm, tsum, -1.0 / D)
    sq = pool.tile([128, D], F32, tag="ln_t1", bufs=2)
    nc.scalar.activation(sq, resid, AF.Square, accum_out=ssq)
    nc.vector.tensor_mul(m2, nm, nm)
    nc.vector.scalar_tensor_tensor(var, ssq, 1.0 / D, m2, ALU.mult, ALU.subtract)
    nc.scalar.activation(stdv, var, AF.Sqrt, bias=eps_ap)
    nc.vector.reciprocal(rstd, stdv)
    t1 = pool.tile([128, D], F32, tag="ln_t1", bufs=2)
    nc.vector.scalar_tensor_tensor(t1, resid, nm, g_b, ALU.add, ALU.mult)
    for spec, dst in ((out1, dst1), (out2, dst2)):
        if spec is None:
            continue
        name, dt_, beta_ap = spec
        o_ = pool.tile([128, D], dt_, tag=name, bufs=1)
        nc.vector.scalar_tensor_tensor(o_, t1, rstd, beta_ap, ALU.mult, ALU.add)
        nc.sync.dma_start(out=dst, in_=o_)


def _build_nc():
    nc = bacc.Bacc("TRN2", target_bir_lowering=False, debug=False,
                   num_devices=NCORES)

    # ---- DRAM I/O ----------------------------------------------------------
    xb = nc.declare_dram_parameter("xb", [TEXT, D], BF16, isOutput=False)
    xr = nc.declare_dram_parameter("xr", [TOWN, D], F32, isOutput=False)
    maskb = nc.declare_dram_parameter("maskb", [128, 2 * W], F32, isOutput=False)
    wqkvT = nc.declare_dram_parameter("wqkvT", [D, 3 * D], BF16, isOutput=False)
    owT = nc.declare_dram_parameter("owT", [D, D], BF16, isOutput=False)
    w1 = nc.declare_dram_parameter("w1", [D, FF], BF16, isOutput=False)
    w2 = nc.declare_dram_parameter("w2", [FF, D], BF16, isOutput=False)
    bqk = nc.declare_dram_parameter("bqk", [128, 2 * KD], F32, isOutput=False)
    bf1p = nc.declare_dram_parameter("bf1p", [128, KF], F32, isOutput=False)
    # const rows: 0=bv 1=gamma1 2=beta1 3=beta1+bf2 4=gamma2 5=beta2
    crow = nc.declare_dram_parameter("crow", [6, D], F32, isOutput=False)
    out = nc.declare_dram_parameter("out", [TOWN, D], F32, isOutput=True)

    # DRAM intermediates
    x1b = nc.dram_tensor("x1b", [TOWN, D], BF16)
    x1r = nc.dram_tensor("x1r", [TOWN, D], F32)
    ht = nc.dram_tensor("ht", [FF, TOWN], BF16)

    wqkvT_r = wqkvT.rearrange("(k p) n -> p k n", p=128)
    owT_r = owT.rearrange("(k p) n -> p k n", p=128)
    w1_r = w1.rearrange("(k p) n -> p k n", p=128)
    w2_r = w2.rearrange("(k p) n -> p k n", p=128)

    def bcast_row(row):
        return bass.AP(tensor=crow, offset=row * D,
                       ap=[[0, 128], [1, D]])

    with tile.TileContext(nc) as tc:
        with tc.tile_pool(name="const", bufs=1) as cpool, \
             tc.tile_pool(name="psum", bufs=2, space="PSUM") as psum:

            # ---- constants (whole-kernel lifetime) -------------------------
            ident = cpool.tile([128, 128], BF16)
            make_identity(nc, ident)
            eps_sb = cpool.tile([128, 1], F32)
            nc.vector.memset(eps_sb, EPS)
            bqk_sb = cpool.tile([128, 2 * KD], F32)
            nc.sync.dma_start(out=bqk_sb, in_=bqk.ap())
            bf1_sb = cpool.tile([128, KF], F32)
            nc.sync.dma_start(out=bf1_sb, in_=bf1p.ap())
            mask_sb = cpool.tile([128, 2 * W], F32)
            nc.sync.dma_start(out=mask_sb, in_=maskb.ap())
            btiles = []
            for i in range(6):
                b_ = cpool.tile([128, D], F32, name=f"crow{i}")
                nc.sync.dma_start(out=b_, in_=bcast_row(i))
                btiles.append(b_)
            bv_b, g1_b, b1_b, b1f2_b, g2_b, b2_b = btiles

            # =================================================================
            # Phase 1: per-chunk QKV + attention + out-proj + LN1
            # =================================================================
            with tc.tile_pool(name="attn", bufs=2) as ap:
                wqkv_sb = ap.tile([128, KD, 3 * D], BF16, bufs=1)
                nc.sync.dma_start(out=wqkv_sb, in_=wqkvT_r)
                ow_sb = ap.tile([128, KD, D], BF16, bufs=1)
                nc.sync.dma_start(out=ow_sb, in_=owT_r)
                kT_sb = ap.tile([128, KD, TEXT], BF16, bufs=1)

                def load_xT(e):
                    """DMA ext chunk e (256 tokens); transpose feature-major."""
                    xT = ap.tile([128, KD, 2 * 128], BF16, tag="xT", bufs=2)
                    for tb in range(2):
                        x_sb = ap.tile([128, D], BF16, tag="x_sb", bufs=2)
                        t0 = e * W + tb * 128
                        nc.sync.dma_start(out=x_sb, in_=xb[t0:t0 + 128, :])
                        pt = psum.tile([128, KD, 128], BF16, tag="tr")
                        for d_ in range(KD):
                            nc.tensor.transpose(
                                pt[:, d_, :], x_sb[:, d_ * 128:(d_ + 1) * 128],
                                ident)
                        nc.vector.tensor_copy(
                            xT[:, :, tb * 128:(tb + 1) * 128], pt)
                    return xT

                def qkv_chunk(e, xT, v_t, qT=None):
                    """Project ext chunk e into kT_sb cols / v_t / (q) qT."""
                    for which in range(2):      # 0 = q, 1 = k
                        if which == 0 and qT is None:
                            continue
                        for m in range(KD):
                            ps = psum.tile([128, W], F32, tag="mm")
                            for k in range(KD):
                                nc.tensor.matmul(
                                    ps,
                                    wqkv_sb[:, k, which * D + m * 128:
                                            which * D + (m + 1) * 128],
                                    xT[:, k, :],
                                    start=(k == 0), stop=(k == KD - 1))
                            if which == 0:
                                nc.vector.tensor_scalar_add(
                                    qT[:, m, :], ps, bqk_sb[:, m:m + 1])
                            else:
                                nc.vector.tensor_scalar_add(
                                    kT_sb[:, m, e * W:(e + 1) * W], ps,
                                    bqk_sb[:, KD + m:KD + m + 1])
                    for tb in range(2):         # v, token-major
                        for n in range(2):
                            ps = psum.tile([128, 512], F32, tag="mm")
                            for k in range(KD):
                                nc.tensor.matmul(
                                    ps, xT[:, k, tb * 128:(tb + 1) * 128],
                                    wqkv_sb[:, k, 2 * D + n * 512:
                                            2 * D + (n + 1) * 512],
                                    start=(k == 0), stop=(k == KD - 1))
                            nc.vector.scalar_tensor_tensor(
                                v_t[:, tb, n * 512:(n + 1) * 512], ps, 1.0,
                                bv_b[:, n * 512:(n + 1) * 512],
                                ALU.mult, ALU.add)

                v_tiles = []
                xT0 = load_xT(0)
                v0 = ap.tile([128, 2, D], BF16, tag="v", bufs=3)
                qkv_chunk(0, xT0, v0)
                v_tiles.append(v0)

                for c in range(NCH):
                    e = c + 1
                    xT = load_xT(e)
                    qT = ap.tile([128, KD, W], BF16, tag="qT", bufs=2)
                    v_t = ap.tile([128, 2, D], BF16, tag="v", bufs=3)
                    qkv_chunk(e, xT, v_t, qT)
                    v_tiles.append(v_t)
                    v_prev, v_cur = v_tiles[c], v_tiles[c + 1]

                    # ---- attention --------------------------------------
                    oT = ap.tile([128, KD, W], BF16, tag="oT", bufs=2)
                    for hp in range(H // 2):
                        attnT = [None, None]
                        for hh in range(2):
                            aT = ap.tile([128, 4, W], BF16, tag="attnT", bufs=3)
                            attnT[hh] = aT
                            for qt in range(2):
                                sc = psum.tile([128, 2 * W], F32, tag="score")
                                nc.tensor.matmul(
                                    sc,
                                    qT[hh * 64:(hh + 1) * 64, hp,
                                       qt * 128:(qt + 1) * 128],
                                    kT_sb[hh * 64:(hh + 1) * 64, hp,
                                          c * W:(c + 2) * W],
                                    start=True, stop=True,
                                    tile_position=(hh * 64, 0))
                                if c == 0:
                                    nc.vector.tensor_add(sc, sc, mask_sb)
                                es = ap.tile([128, 2 * W], F32, tag="es",
                                             bufs=2)
                                sr = ap.tile([128, 2], F32, tag="sr", bufs=4)
                                nc.scalar.activation(es, sc, AF.Exp,
                                                     scale=SCALE,
                                                     accum_out=sr[:, 0:1])
                                nc.vector.reciprocal(sr[:, 1:2], sr[:, 0:1])
                                ab = ap.tile([128, 2 * W], BF16, tag="ab",
                                             bufs=2)
                                nc.vector.tensor_scalar_mul(ab, es, sr[:, 1:2])
                                pt = psum.tile([128, 4, 128], BF16, tag="tr")
                                for i in range(4):
                                    nc.tensor.transpose(
                                        pt[:, i, :],
                                        ab[:, i * 128:(i + 1) * 128], ident)
                                if (qt + hh) % 2 == 0:
                                    nc.vector.tensor_copy(
                                        aT[:, :, qt * 128:(qt + 1) * 128], pt)
                                else:
                                    nc.scalar.copy(
                                        aT[:, :, qt * 128:(qt + 1) * 128], pt)
                        ot = psum.tile([128, W], F32, tag="ot")
                        for i in range(4):
                            v_src = v_prev if i < 2 else v_cur
                            tb = i % 2
                            for hh in range(2):
                                h = hp * 2 + hh
                                nc.tensor.matmul(
                                    ot[hh * 64:(hh + 1) * 64, :],
                                    v_src[:, tb, h * 64:(h + 1) * 64],
                                    attnT[hh][:, i, :],
                                    start=(i == 0), stop=(i == 3),
                                    tile_position=(0, hh * 64))
                        if hp % 2 == 0:
                            nc.vector.tensor_copy(oT[:, hp, :], ot)
                        else:
                            nc.scalar.copy(oT[:, hp, :], ot)

                    # ---- out-proj + residual + LN1 ----------------------
                    for mt in range(2):
                        resid = ap.tile([128, D], F32, tag="resid", bufs=1)
                        xr_sb = ap.tile([128, D], F32, tag="xr_sb", bufs=1)
                        nc.sync.dma_start(
                            out=xr_sb,
                            in_=xr[c * W + mt * 128:c * W + (mt + 1) * 128, :])
                        for n in range(2):
                            op = psum.tile([128, 512], F32, tag="mm")
                            for k in range(KD):
                                nc.tensor.matmul(
                                    op, oT[:, k, mt * 128:(mt + 1) * 128],
                                    ow_sb[:, k, n * 512:(n + 1) * 512],
                                    start=(k == 0), stop=(k == KD - 1))
                            nc.vector.tensor_add(
                                resid[:, n * 512:(n + 1) * 512], op,
                                xr_sb[:, n * 512:(n + 1) * 512])
                        r0 = c * W + mt * 128
                        _layernorm(nc, ap, resid, g1_b, eps_sb,
                                   ("x1b_sb", BF16, b1_b),
                                   x1b[r0:r0 + 128, :],
                                   ("x1r_sb", F32, b1f2_b),
                                   x1r[r0:r0 + 128, :])

            # =================================================================
            # Phase 2: FFN pass 1  (h = relu(x1 @ w1 + bf1) -> ht)
            # =================================================================
            with tc.tile_pool(name="ffn1", bufs=2) as fp:
                w1_sb = fp.tile([128, KD, FF], BF16, bufs=1)
                nc.sync.dma_start(out=w1_sb, in_=w1_r)
                for tt in range(NCH):
                    x1T = fp.tile([128, KD, W], BF16, tag="x1T", bufs=2)
                    for tb in range(2):
                        x1_sb = fp.tile([128, D], BF16, tag="x1_sb", bufs=3)
                        t0 = tt * W + tb * 128
                        nc.sync.dma_start(out=x1_sb, in_=x1b[t0:t0 + 128, :])
                        pt = psum.tile([128, KD, 128], BF16, tag="tr")
                        for d_ in range(KD):
                            nc.tensor.transpose(
                                pt[:, d_, :], x1_sb[:, d_ * 128:(d_ + 1) * 128],
                                ident)
                        nc.vector.tensor_copy(
                            x1T[:, :, tb * 128:(tb + 1) * 128], pt)
                    for fb in range(8):
                        hT = fp.tile([128, 4, W], BF16, tag="hT", bufs=3)
                        for i in range(4):
                            ps = psum.tile([128, W], F32, tag="mm")
                            for k in range(KD):
                                nc.tensor.matmul(
                                    ps,
                                    w1_sb[:, k, fb * 512 + i * 128:
                                          fb * 512 + (i + 1) * 128],
                                    x1T[:, k, :],
                                    start=(k == 0), stop=(k == KD - 1))
                            nc.scalar.activation(
                                hT[:, i, :], ps, AF.Relu,
                                bias=bf1_sb[:, fb * 4 + i:fb * 4 + i + 1])
                        nc.sync.dma_start(
                            out=ht[fb * 512:(fb + 1) * 512,
                                   tt * W:(tt + 1) * W]
                            .rearrange("(i p) t -> p i t", p=128),
                            in_=hT)

            # =================================================================
            # Phase 3: FFN pass 2  (out = LN2(x1 + h @ w2 + bf2))
            # =================================================================
            with tc.tile_pool(name="ffn2", bufs=2) as gp:
                w2_sb = gp.tile([128, KF, D], BF16, bufs=1)
                nc.sync.dma_start(out=w2_sb, in_=w2_r)
                for tt in range(NCH):
                    acc = [[None] * 2 for _ in range(2)]
                    for mt in range(2):
                        for n in range(2):
                            acc[mt][n] = psum.tile(
                                [128, 512], F32, tag=("mm", "score")[mt],
                                name=f"acc{mt}{n}")
                    for fb in range(8):
                        ht_in = gp.tile([128, 4, W], BF16, tag="ht_in", bufs=4)
                        nc.sync.dma_start(
                            out=ht_in,
                            in_=ht[fb * 512:(fb + 1) * 512,
                                   tt * W:(tt + 1) * W]
                            .rearrange("(i p) t -> p i t", p=128))
                        for mt in range(2):
                            for n in range(2):
                                for i in range(4):
                                    nc.tensor.matmul(
                                        acc[mt][n],
                                        ht_in[:, i, mt * 128:(mt + 1) * 128],
                                        w2_sb[:, fb * 4 + i,
                                              n * 512:(n + 1) * 512],
                                        start=(fb == 0 and i == 0),
                                        stop=(fb == 7 and i == 3))
                    for mt in range(2):
                        resid = gp.tile([128, D], F32, tag="resid2", bufs=2)
                        xr1_sb = gp.tile([128, D], F32, tag="xr1_sb", bufs=2)
                        t0 = tt * W + mt * 128
                        nc.sync.dma_start(out=xr1_sb, in_=x1r[t0:t0 + 128, :])
                        for n in range(2):
                            nc.vector.tensor_add(
                                resid[:, n * 512:(n + 1) * 512], acc[mt][n],
                                xr1_sb[:, n * 512:(n + 1) * 512])
                        _layernorm(nc, gp, resid, g2_b, eps_sb,
                                   ("out_sb", F32, b2_b), out[t0:t0 + 128, :])

    nc.compile()
    return nc


_NC_CACHE = None


def _get_nc():
    global _NC_CACHE
    if _NC_CACHE is None:
        _NC_CACHE = _build_nc()
    return _NC_CACHE


def kernel(x, in_proj_w, in_proj_b, out_w, out_b, gamma1, beta1,
           w1, bf1, w2, bf2, gamma2, beta2):
    bf = ml_dtypes.bfloat16
    f32 = np.float32
    x = np.asarray(x, f32)
    B, L, _ = x.shape
    xf = x.reshape(B * L, D)
    in_proj_w = np.asarray(in_proj_w, f32)
    in_proj_b = np.asarray(in_proj_b, f32)
    wq, wk, wv = in_proj_w[:D], in_proj_w[D:2 * D], in_proj_w[2 * D:]
    bq, bk, bv = in_proj_b[:D], in_proj_b[D:2 * D], in_proj_b[2 * D:]

    wqkvT = np.concatenate([wq.T, wk.T, wv.T], axis=1).astype(bf)  # [D, 3D]
    owT = np.asarray(out_w, f32).T.astype(bf)
    w1b = np.asarray(w1, f32).astype(bf)
    w2b = np.asarray(w2, f32).astype(bf)
    bqk_h = np.concatenate(
        [bq.reshape(KD, 128).T, bk.reshape(KD, 128).T], axis=1).astype(f32)
    bf1_h = np.asarray(bf1, f32).reshape(KF, 128).T.copy()
    crow_h = np.stack([
        bv,
        np.asarray(gamma1, f32),
        np.asarray(beta1, f32),
        np.asarray(beta1, f32) + np.asarray(bf2, f32),
        np.asarray(gamma2, f32),
        np.asarray(beta2, f32),
    ]).astype(f32)

    in_maps = []
    for c in range(NCORES):
        own = xf[c * TOWN:(c + 1) * TOWN]
        mb = np.zeros((128, 2 * W), f32)
        if c % 2 == 0:
            halo = np.zeros((W, D), f32)
            mb[:, :W] = -1e9
        else:
            halo = xf[c * TOWN - W:c * TOWN]
        in_maps.append({
            "xb": np.concatenate([halo, own]).astype(bf),
            "xr": (own + np.asarray(out_b, f32)[None, :]).astype(f32),
            "maskb": mb,
            "wqkvT": wqkvT,
            "owT": owT,
            "w1": w1b,
            "w2": w2b,
            "bqk": bqk_h,
            "bf1p": bf1_h,
            "crow": crow_h,
        })

    nc = _get_nc()
    res = run_bass_kernel_spmd(nc, in_maps, list(range(NCORES)))
    outs = np.concatenate([res.results[c]["out"] for c in range(NCORES)], axis=0)
    return outs.reshape(B, L, D).astype(f32)
